# revision 1
# baseline (speedup 1.0000x reference)
"""Trainium2 Bass kernel for LocalHistogramLayer (histogram_binning).

Math (reference):
    d[n,o]   = ||x_n - c_o||^2
    rbf      = exp(-d/2)
    hist[o,i]= sum_n rbf[n,o] * x[n,i]

Device strategy (8 cores, data-parallel over N):
  Per core (N_loc = 65536), chunks of 512 points:
    G1:  psum[-d/2][o=128, n=512] = CT_ext.T @ XT_ext        (one fp32r matmul,
         CT_ext stationary; contraction K=66 = 64 features + 2 aux rows that
         fold in -x2/2 and -c2/2, so PSUM holds exactly -d/2)
    exp: ACT Exp, PSUM -> SBUF [o, n]
    T:   4x PE transpose (128x128) -> PSUM, one DVE copy -> SBUF  (rbf^T [n,o])
    G2:  4x fp32r matmul accumulating hist[o=128, i=64] in PSUM
  Host: ships x^T(+aux rows) in a per-chunk column permutation so the natural
  x load is contiguous per partition; sums the 8 per-core partial histograms.
"""

import sys

if "/opt/trn_rl_repo" not in sys.path:
    sys.path.insert(0, "/opt/trn_rl_repo")

import numpy as np

import concourse.bass as bass
import concourse.bacc as bacc
import concourse.mybir as mybir
import concourse.tile as tile

N_TOTAL = 524288
IN = 64
OUT = 128
NCORES = 8
NLOC = N_TOTAL // NCORES  # 65536
CHUNK = 512
KEXT = IN + 4  # 68: features + (-x2/2) hi/lo rows + two ones rows for c2 hi/lo


def _split10(v):
    """hi keeps 10 mantissa bits (exactly representable in any fp32r
    variant with >=10-bit mantissa, so the PE rounds it losslessly)."""
    v = np.asarray(v, np.float32)
    hi = (v.view(np.uint32) & np.uint32(0xFFFFE000)).view(np.float32)
    return hi, (v - hi).astype(np.float32)

F32 = mybir.dt.float32
F32R = mybir.dt.float32r


def build_nc(nloc=NLOC, chunk=CHUNK, use_f32r=True, f32r_transpose=True):
    nchunks = nloc // chunk
    nsub = chunk // 128  # 128-point sub-tiles per chunk

    nc = bacc.Bacc("TRN2", target_bir_lowering=False, debug=False)

    mm_dt = F32R if use_f32r else F32
    tr_dt = F32R if f32r_transpose else F32

    # The BIR verifier requires every producer feeding an FP32r matmul to
    # emit float32r, so the whole matmul datapath (DRAM -> SBUF -> PSUM-T)
    # is declared float32r. np-dtype is still float32 (same bits).
    xt_d = nc.dram_tensor("xt_ext", [KEXT, nloc], mm_dt, kind="ExternalInput")
    xn_d = nc.dram_tensor("x_nat", [nloc, IN], mm_dt, kind="ExternalInput")
    ct_d = nc.dram_tensor("ct_ext", [KEXT, OUT], mm_dt, kind="ExternalInput")
    id_d = nc.dram_tensor("ident", [128, 128], tr_dt, kind="ExternalInput")
    out_d = nc.dram_tensor("hist_out", [OUT, IN], F32, kind="ExternalOutput")

    with tile.TileContext(nc) as tc:
        with (
            tc.tile_pool(name="const", bufs=1) as const_pool,
            tc.tile_pool(name="xt", bufs=6) as xt_pool,
            tc.tile_pool(name="xn", bufs=6) as xn_pool,
            tc.tile_pool(name="rbf", bufs=3) as rbf_pool,
            tc.tile_pool(name="rbft", bufs=6) as rbft_pool,
            tc.tile_pool(name="ps_g1", bufs=2, space="PSUM") as ps_g1_pool,
            tc.tile_pool(name="ps_t", bufs=3, space="PSUM") as ps_t_pool,
            tc.tile_pool(name="ps_h", bufs=1, space="PSUM") as ps_h_pool,
        ):
            ct_sb = const_pool.tile([KEXT, OUT], mm_dt)
            nc.sync.dma_start(ct_sb[:], ct_d[:])
            id_sb = const_pool.tile([128, 128], tr_dt)
            nc.sync.dma_start(id_sb[:], id_d[:])

            hist_ps = ps_h_pool.tile([OUT, IN], F32)

            npair = nchunks // 2
            for pr in range(npair):
                # pair of chunks shares one PSUM-G1 tile (2 banks) and a
                # single [128, 1024] exp, halving ACT op count + sync points
                g1_ps = ps_g1_pool.tile([OUT, 2, chunk], F32)
                rbf_sb = rbf_pool.tile([OUT, 2, chunk], tr_dt)
                xn_sbs = []
                for j in range(2):
                    c = 2 * pr + j
                    n0 = c * chunk
                    xt_sb = xt_pool.tile([KEXT, chunk], mm_dt)
                    nc.sync.dma_start(xt_sb[:], xt_d[:, n0 : n0 + chunk])
                    xn_sb = xn_pool.tile([128, nsub, IN], mm_dt)
                    nc.sync.dma_start(
                        xn_sb[:],
                        xn_d[n0 : n0 + chunk, :].rearrange(
                            "(p k) i -> p k i", k=nsub
                        ),
                    )
                    xn_sbs.append(xn_sb)
                    nc.tensor.matmul(
                        g1_ps[:, j, :],
                        ct_sb[:],
                        xt_sb[:],
                        start=True,
                        stop=True,
                    )

                nc.scalar.activation(
                    rbf_sb[:], g1_ps[:], mybir.ActivationFunctionType.Exp
                )

                for j in range(2):
                    c = 2 * pr + j
                    t_ps = ps_t_pool.tile([128, nsub, 128], tr_dt)
                    for k in range(nsub):
                        nc.tensor.matmul(
                            t_ps[:, k, :],
                            rbf_sb[:, j, k * 128 : (k + 1) * 128],
                            id_sb[:],
                            is_transpose=True,
                            start=(k == 0),
                            stop=(k == nsub - 1),
                        )
                    rbft_sb = rbft_pool.tile([128, nsub, 128], mm_dt)
                    nc.vector.tensor_copy(rbft_sb[:], t_ps[:])
                    for k in range(nsub):
                        nc.tensor.matmul(
                            hist_ps[:],
                            rbft_sb[:, k, :],
                            xn_sbs[j][:, k, :],
                            start=(c == 0 and k == 0),
                            stop=(c == nchunks - 1 and k == nsub - 1),
                        )

            hist_sb = const_pool.tile([OUT, IN], F32)
            nc.vector.tensor_copy(hist_sb[:], hist_ps[:])
            nc.sync.dma_start(out_d[:], hist_sb[:])

    nc.compile()
    return nc


def make_host_inputs(x, bin_centers, nloc=NLOC, chunk=CHUNK, ncores=NCORES):
    """Build per-core input maps. Host-side numpy prep (not device-timed)."""
    x = np.ascontiguousarray(x, dtype=np.float32)
    c = np.ascontiguousarray(bin_centers, dtype=np.float32)

    c2 = np.sum(c.astype(np.float64) * c, axis=1).astype(np.float32)  # [OUT]
    c2hi, c2lo = _split10(-0.5 * c2)
    ct_ext = np.empty((KEXT, OUT), dtype=np.float32)
    ct_ext[:IN] = c.T
    ct_ext[IN] = 1.0  # pairs with -x2/2 hi row
    ct_ext[IN + 1] = 1.0  # pairs with -x2/2 lo row
    ct_ext[IN + 2] = c2hi  # pairs with ones row
    ct_ext[IN + 3] = c2lo  # pairs with ones row

    ident = np.eye(128, dtype=np.float32)

    nchunks = nloc // chunk
    nsub = chunk // 128
    in_maps = []
    for core in range(ncores):
        xs = x[core * nloc : (core + 1) * nloc]  # [nloc, IN]
        x2 = np.sum(xs * xs, axis=1)  # [nloc] fp32, matches on-device style

        # Per-chunk column permutation: xt column (128k + p) must hold point
        # n0 + nsub*p + k, matching the natural-load layout (partition p gets
        # nsub consecutive rows) after the PE transpose of rbf.
        xs4 = xs.reshape(nchunks, 128, nsub, IN)
        xt_x = xs4.transpose(3, 0, 2, 1).reshape(IN, nloc)
        x24 = x2.reshape(nchunks, 128, nsub)
        xt_x2 = x24.transpose(0, 2, 1).reshape(nloc)

        x2hi, x2lo = _split10(-0.5 * xt_x2)
        xt_ext = np.empty((KEXT, nloc), dtype=np.float32)
        xt_ext[:IN] = xt_x
        xt_ext[IN] = x2hi
        xt_ext[IN + 1] = x2lo
        xt_ext[IN + 2] = 1.0
        xt_ext[IN + 3] = 1.0

        in_maps.append(
            {
                "xt_ext": np.ascontiguousarray(xt_ext),
                "x_nat": np.ascontiguousarray(xs),
                "ct_ext": ct_ext,
                "ident": ident,
            }
        )
    return in_maps


_CACHED_NC = None


def _get_nc():
    global _CACHED_NC
    if _CACHED_NC is None:
        _CACHED_NC = build_nc()
    return _CACHED_NC


def run_on_hw(in_maps, trace=False, **kwargs):
    from concourse.bass_utils import run_bass_kernel_spmd

    nc = _get_nc()
    return run_bass_kernel_spmd(
        nc, in_maps, list(range(len(in_maps))), trace=trace, **kwargs
    )


def kernel(x, bin_centers):
    in_maps = make_host_inputs(x, bin_centers)
    res = run_on_hw(in_maps)
    parts = np.stack([r["hist_out"] for r in res.results])  # [8, OUT, IN]
    return np.sum(parts, axis=0, dtype=np.float64).astype(np.float32)



# revision 3
# speedup vs baseline: 3.3712x; 3.3712x over previous
"""Trainium2 Bass kernel for LocalHistogramLayer (histogram_binning).

Math (reference):
    d[n,o]   = ||x_n - c_o||^2
    rbf      = exp(-d/2)
    hist[o,i]= sum_n rbf[n,o] * x[n,i]

Factorization used here:
    rbf[n,o] = exp(x_n.c_o - ||c_o||^2/2) * exp(-||x_n||^2/2)
             =        E[n,o]              *       a[n]
    hist[o,i]= sum_n E[n,o] * (a[n] * x[n,i]) = E^T @ (a*x)

Device strategy (8 cores, data-parallel over N). The dominant cost in this
environment is host->device transfer over the axon tunnel (~77 MB/s, with a
severe cliff above ~200 MB total). So the kernel ships x exactly ONCE, as
fp16 (8 MB/core, 64 MB total), and builds everything else on device:

  Per core (N_loc = 65536), chunks of 512 points:
    up:   DVE upcast x16 [128,4,64] -> f32r
    xT:   4x PE transpose ([128,64] -> [64,128] PSUM) + DVE copy -> xt [64,512]
    G1:   psum[o=128, n=512] = ct[64,128].T @ xt  (x.c, K=64 f32r matmul)
    exp:  ACT Exp(psum + bias) with per-partition bias = -||c_o||^2/2 -> E
    a:    DVE square + reduce + ACT exp(-x2/2); y = a*x (4x tensor_scalar)
    T:    4x PE transpose of E -> PSUM, DVE copy -> E^T [n,o]
    G2:   4x f32r matmul accumulating hist[o=128, i=64] in PSUM over all chunks
  Host: sums the 8 per-core partial histograms (fp64) -> fp32.

fp16 on x costs ~2.4e-4 relative noise on x -> ~0.2% on the dominant rbf
values, far inside the 2e-2 gate, and halves the shipped bytes vs fp32.
"""

import sys

if "/opt/trn_rl_repo" not in sys.path:
    sys.path.insert(0, "/opt/trn_rl_repo")

import numpy as np

import concourse.bass as bass
import concourse.bacc as bacc
import concourse.mybir as mybir
import concourse.tile as tile

N_TOTAL = 524288
IN = 64
OUT = 128
NCORES = 8
NLOC = N_TOTAL // NCORES  # 65536
CHUNK = 512
NSUB = CHUNK // 128  # 4

F32 = mybir.dt.float32
F32R = mybir.dt.float32r
F16 = mybir.dt.float16


def build_nc(nloc=NLOC, chunk=CHUNK):
    nchunks = nloc // chunk
    nsub = chunk // 128

    nc = bacc.Bacc("TRN2", target_bir_lowering=False, debug=False)

    x16_d = nc.dram_tensor("x16", [nloc, IN], F16, kind="ExternalInput")
    ct_d = nc.dram_tensor("ct", [IN, OUT], F32R, kind="ExternalInput")
    c2b_d = nc.dram_tensor("c2b", [OUT, 1], F32, kind="ExternalInput")
    id_d = nc.dram_tensor("ident", [128, 128], F32R, kind="ExternalInput")
    out_d = nc.dram_tensor("hist_out", [OUT, IN], F32, kind="ExternalOutput")

    with tile.TileContext(nc) as tc:
        with (
            tc.tile_pool(name="const", bufs=1) as const_pool,
            tc.tile_pool(name="x16", bufs=6) as x16_pool,
            tc.tile_pool(name="x32", bufs=3) as x32_pool,
            tc.tile_pool(name="sq", bufs=3) as sq_pool,
            tc.tile_pool(name="stat", bufs=4) as stat_pool,
            tc.tile_pool(name="y", bufs=3) as y_pool,
            tc.tile_pool(name="xt", bufs=3) as xt_pool,
            tc.tile_pool(name="rbf", bufs=3) as rbf_pool,
            tc.tile_pool(name="rbft", bufs=3) as rbft_pool,
            tc.tile_pool(name="ps_xt", bufs=2, space="PSUM") as ps_xt_pool,
            tc.tile_pool(name="ps_g1", bufs=2, space="PSUM") as ps_g1_pool,
            tc.tile_pool(name="ps_t", bufs=2, space="PSUM") as ps_t_pool,
            tc.tile_pool(name="ps_h", bufs=1, space="PSUM") as ps_h_pool,
        ):
            ct_sb = const_pool.tile([IN, OUT], F32R)
            nc.sync.dma_start(ct_sb[:], ct_d[:])
            c2b_sb = const_pool.tile([OUT, 1], F32)
            nc.sync.dma_start(c2b_sb[:], c2b_d[:])
            id_sb = const_pool.tile([128, 128], F32R)
            nc.sync.dma_start(id_sb[:], id_d[:])

            hist_ps = ps_h_pool.tile([OUT, IN], F32)

            for c in range(nchunks):
                n0 = c * chunk
                # natural load: partition p, slot k holds point n0 + nsub*p + k
                x16_sb = x16_pool.tile([128, nsub, IN], F16)
                nc.sync.dma_start(
                    x16_sb[:],
                    x16_d[n0 : n0 + chunk, :].rearrange(
                        "(p k) i -> p k i", k=nsub
                    ),
                )
                x32_sb = x32_pool.tile([128, nsub, IN], F32R)
                nc.vector.tensor_copy(x32_sb[:], x16_sb[:])

                # xt[i, 128k+p] = x[n0+nsub*p+k, i] via 4 PE transposes
                xt_ps = ps_xt_pool.tile([IN, nsub, 128], F32R)
                for k in range(nsub):
                    nc.tensor.matmul(
                        xt_ps[:, k, :],
                        x32_sb[:, k, :],
                        id_sb[:],
                        is_transpose=True,
                        start=(k == 0),
                        stop=(k == nsub - 1),
                    )
                xt_sb = xt_pool.tile([IN, nsub, 128], F32R)
                nc.vector.tensor_copy(xt_sb[:], xt_ps[:])

                # G1: xc[o, col] ; exp(xc - c2/2) -> E
                g1_ps = ps_g1_pool.tile([OUT, chunk], F32)
                nc.tensor.matmul(
                    g1_ps[:],
                    ct_sb[:],
                    xt_sb[:].rearrange("i k p -> i (k p)"),
                    start=True,
                    stop=True,
                )
                rbf_sb = rbf_pool.tile([OUT, chunk], F32R)
                nc.scalar.activation(
                    rbf_sb[:],
                    g1_ps[:],
                    mybir.ActivationFunctionType.Exp,
                    bias=c2b_sb[:],
                )

                # a = exp(-x2/2); y = a * x
                sq_sb = sq_pool.tile([128, nsub, IN], F32)
                nc.vector.tensor_tensor(
                    sq_sb[:], x32_sb[:], x32_sb[:], mybir.AluOpType.mult
                )
                x2_sb = stat_pool.tile([128, nsub], F32)
                nc.vector.tensor_reduce(
                    x2_sb[:], sq_sb[:], mybir.AxisListType.X, mybir.AluOpType.add
                )
                a_sb = stat_pool.tile([128, nsub], F32)
                nc.scalar.activation(
                    a_sb[:],
                    x2_sb[:],
                    mybir.ActivationFunctionType.Exp,
                    scale=-0.5,
                )
                y_sb = y_pool.tile([128, nsub, IN], F32R)
                for k in range(nsub):
                    nc.vector.tensor_scalar_mul(
                        y_sb[:, k, :], x32_sb[:, k, :], a_sb[:, k : k + 1]
                    )

                # transpose E -> E^T [point, o] (partition p, slot k)
                t_ps = ps_t_pool.tile([128, nsub, 128], F32R)
                for k in range(nsub):
                    nc.tensor.matmul(
                        t_ps[:, k, :],
                        rbf_sb[:, k * 128 : (k + 1) * 128],
                        id_sb[:],
                        is_transpose=True,
                        start=(k == 0),
                        stop=(k == nsub - 1),
                    )
                rbft_sb = rbft_pool.tile([128, nsub, 128], F32R)
                nc.vector.tensor_copy(rbft_sb[:], t_ps[:])

                # G2: hist[o, i] += sum_n E^T[n, o] * y[n, i]
                for k in range(nsub):
                    nc.tensor.matmul(
                        hist_ps[:],
                        rbft_sb[:, k, :],
                        y_sb[:, k, :],
                        start=(c == 0 and k == 0),
                        stop=(c == nchunks - 1 and k == nsub - 1),
                    )

            hist_sb = const_pool.tile([OUT, IN], F32)
            nc.vector.tensor_copy(hist_sb[:], hist_ps[:])
            nc.sync.dma_start(out_d[:], hist_sb[:])

    nc.compile()
    return nc


def make_host_inputs(x, bin_centers, nloc=NLOC, ncores=NCORES):
    """Build per-core input maps. Host-side numpy prep (not device-timed)."""
    x = np.ascontiguousarray(x, dtype=np.float32)
    c = np.ascontiguousarray(bin_centers, dtype=np.float32)

    ct = np.ascontiguousarray(c.T)  # [IN, OUT] f32
    c2 = np.sum(c.astype(np.float64) * c, axis=1)  # [OUT]
    c2b = np.ascontiguousarray((-0.5 * c2)[:, None].astype(np.float32))
    ident = np.eye(128, dtype=np.float32)

    in_maps = []
    for core in range(ncores):
        x16 = np.ascontiguousarray(
            x[core * nloc : (core + 1) * nloc].astype(np.float16)
        )
        in_maps.append(
            {"x16": x16, "ct": ct, "c2b": c2b, "ident": ident}
        )
    return in_maps


_CACHED_NC = None


def _get_nc():
    global _CACHED_NC
    if _CACHED_NC is None:
        _CACHED_NC = build_nc()
    return _CACHED_NC


def run_on_hw(in_maps, trace=False, **kwargs):
    from concourse.bass_utils import run_bass_kernel_spmd

    nc = _get_nc()
    return run_bass_kernel_spmd(
        nc, in_maps, list(range(len(in_maps))), trace=trace, **kwargs
    )


def kernel(x, bin_centers):
    in_maps = make_host_inputs(x, bin_centers)
    res = run_on_hw(in_maps)
    parts = np.stack([r["hist_out"] for r in res.results])  # [8, OUT, IN]
    return np.sum(parts, axis=0, dtype=np.float64).astype(np.float32)
